# revision 35
# baseline (speedup 1.0000x reference)
"""Multi-head self-attention (B=4, S=2048, D=1024, H=16, causal) on 8 TRN2
NeuronCores, tensor-parallel over heads (2 heads per core).

Per-core (bf16 matmuls, fp32 PSUM):
  qkv projection (channel-major) -> v PE-transposed to [s,dk]+ones col ->
  attention with TRANSPOSED scores ([k,q] layout; no probs transposes;
  exp on ScalarE over a fused h0|h1 [128,1024] PSUM pair; causal skip +
  triangle mask on diagonal tiles; PV accumulates [65,q]=[v|1]^T@expT,
  row 64 = softmax denominator; normalization on [64,q] head outputs) ->
  output projection partial per q-block; host sums 8 partials.

Emission order software-pipelines: next batch's QKV/v-transposes fill the
attention loop, each q-block's projection follows its epilogue -- keeps
TensorE dense so HAM stays at K=8/8.
"""

import numpy as np
import ml_dtypes

import concourse.bacc as bacc
import concourse.mybir as mybir
import concourse.tile as tile
from concourse import bass_utils

B, S, D, H, DK = 4, 2048, 1024, 16, 64
NCORES = 8
HPC = H // NCORES          # heads per core = 2
CH = HPC * DK              # qkv channels per head group = 128
NQ = 512                   # q block
KT = 128                   # k tile
NQB = S // NQ              # 4 q-blocks per batch
NKT = S // KT              # 16 k-tiles per batch
KD = D // 128              # 8 contraction tiles for projections

F32 = mybir.dt.float32
BF16 = mybir.dt.bfloat16

_CACHE = {}


def _build():
    nc = bacc.Bacc("TRN2", target_bir_lowering=False, debug=False)

    xt_d = nc.dram_tensor("xt", [B, KD, 128, S], BF16, kind="ExternalInput")
    wqkvt_d = nc.dram_tensor("wqkvt", [KD, 128, 3 * CH], BF16, kind="ExternalInput")
    wot_d = nc.dram_tensor("wot", [CH, D], BF16, kind="ExternalInput")
    tri_d = nc.dram_tensor("tri", [128, 128], BF16, kind="ExternalInput")
    iden_d = nc.dram_tensor("iden", [128, 128], BF16, kind="ExternalInput")
    outt_d = nc.dram_tensor("outt", [D // 128, 128, B * S], F32, kind="ExternalOutput")

    with tile.TileContext(nc) as tc:
        with (
            tc.tile_pool(name="xt", bufs=2 * KD) as xt_pool,
            tc.tile_pool(name="qk", bufs=2) as qk_pool,
            tc.tile_pool(name="vst", bufs=2) as vst_pool,
            tc.tile_pool(name="vext", bufs=2 * NKT) as vext_pool,
            tc.tile_pool(name="expt", bufs=6) as expt_pool,
            tc.tile_pool(name="heads", bufs=2) as heads_pool,
            tc.tile_pool(name="wp", bufs=1) as w_pool,
            tc.tile_pool(name="outs", bufs=6) as out_pool,
            tc.tile_pool(name="small", bufs=4) as small_pool,
            tc.tile_pool(name="ppa", bufs=2, space="PSUM") as pp_a,
            tc.tile_pool(name="ppsc", bufs=2, space="PSUM") as pp_sc,
            tc.tile_pool(name="pppv", bufs=2, space="PSUM") as pp_pv,
        ):
            # --- persistent weights / constants ---
            # Only wq[k=0] is DMA'd up front: the first QKV matmul needs just
            # wq[0] + xt[0], so the remaining weight loads are interleaved
            # after the batch-0 x tiles (see gen_A) to cut the startup stall.
            wq = w_pool.tile([128, KD * 3 * CH], BF16, name="wq")
            nc.sync.dma_start(out=wq[:, 0 : 3 * CH], in_=wqkvt_d.ap()[0])
            wot = w_pool.tile([128, D], BF16, name="wot")
            tri = w_pool.tile([128, 128], BF16, name="tri")
            iden = w_pool.tile([128, 128], BF16, name="iden")

            st = {}

            def gen_A(b):
                """QKV projection + v transposes for batch b. Yields ~20x."""
                xts = []
                for k in range(KD):
                    xtk = xt_pool.tile([128, S], BF16, name=f"xt_{b}_{k}", tag="xt")
                    nc.sync.dma_start(out=xtk, in_=xt_d.ap()[b, k])
                    xts.append(xtk)
                    if b == 0 and k < KD - 1:
                        nc.sync.dma_start(
                            out=wq[:, (k + 1) * 3 * CH : (k + 2) * 3 * CH],
                            in_=wqkvt_d.ap()[k + 1],
                        )
                if b == 0:
                    nc.sync.dma_start(out=wot, in_=wot_d.ap())
                    nc.sync.dma_start(out=tri, in_=tri_d.ap())
                    nc.sync.dma_start(out=iden, in_=iden_d.ap())
                qT = qk_pool.tile([128, S], BF16, name=f"q_{b}", tag="q")
                kTt = qk_pool.tile([128, S], BF16, name=f"k_{b}", tag="k")
                vT = vst_pool.tile([128, S], BF16, name=f"v_{b}", tag="v")
                st[b] = {"q": qT, "k": kTt, "v": vT, "ve": []}
                dests = [qT, kTt, vT]
                for m in range(3):
                    for n in range(S // 512):
                        ps = pp_a.tile([128, 512], F32, name=f"qkv_{b}_{m}_{n}", tag="mm")
                        for k in range(KD):
                            nc.tensor.matmul(
                                ps,
                                wq[:, k * 3 * CH + m * CH : k * 3 * CH + (m + 1) * CH],
                                xts[k][:, n * 512 : (n + 1) * 512],
                                start=(k == 0),
                                stop=(k == KD - 1),
                            )
                            if k == KD // 2 - 1:
                                yield
                        dst = dests[m][:, n * 512 : (n + 1) * 512]
                        if n % 2 == 0:
                            nc.scalar.copy(dst, ps)
                        else:
                            nc.vector.tensor_copy(out=dst, in_=ps)
                        yield
                for t in range(NKT):
                    pt = pp_a.tile([128, 128], BF16, name=f"vt_{b}_{t}", tag="mm")
                    nc.tensor.transpose(pt, vT[:, t * 128 : (t + 1) * 128], iden)
                    ve = vext_pool.tile([128, 130], BF16, name=f"vext_{b}_{t}", tag="vext")
                    nc.gpsimd.memset(ve[:, 64:65], 1.0)
                    nc.gpsimd.memset(ve[:, 129:130], 1.0)
                    nc.vector.tensor_copy(out=ve[:, 0:64], in_=pt[:, 0:64])
                    nc.vector.tensor_copy(out=ve[:, 65:129], in_=pt[:, 64:128])
                    st[b]["ve"].append(ve)
                    yield

            def gen_proj(b, n):
                """Output projection for columns [n*512,(n+1)*512) of batch b."""
                headsT = st[b]["h"]
                for m in range(D // 128):
                    ps = pp_a.tile([128, 512], F32, name=f"proj_{b}_{m}_{n}", tag="mm")
                    nc.tensor.matmul(
                        ps,
                        wot[:, m * 128 : (m + 1) * 128],
                        headsT[:, n * 512 : (n + 1) * 512],
                        start=True, stop=True,
                    )
                    ot = out_pool.tile([128, 512], F32, name=f"out_{b}_{m}_{n}", tag="o")
                    nc.vector.tensor_copy(out=ot, in_=ps)
                    nc.sync.dma_start(
                        out=outt_d.ap()[m, :, b * S + n * 512 : b * S + (n + 1) * 512],
                        in_=ot,
                    )
                    yield

            def advance(fillers):
                for g in list(fillers):
                    try:
                        next(g)
                        return
                    except StopIteration:
                        fillers.remove(g)

            def run_C(b, fillers):
                """Attention for batch b, fillers interleaved per unit."""
                qT, kTt = st[b]["q"], st[b]["k"]
                vext = st[b]["ve"]
                headsT = heads_pool.tile([128, S], BF16, name=f"heads_{b}", tag="h")
                st[b]["h"] = headsT
                # longest q-block first: its deep unit loop overlaps best with
                # plentiful filler, and the batch tail ends on the short block
                for qi in range(NQB - 1, -1, -1):
                    pv0 = pp_pv.tile([128, 512], F32, name=f"pv0_{b}_{qi}", tag="pv")
                    pv1 = pp_pv.tile([128, 512], F32, name=f"pv1_{b}_{qi}", tag="pv")
                    jmax = (qi + 1) * (NQ // KT)

                    def emit_pv(j, e, n0):
                        last = j == jmax - 1
                        nc.tensor.matmul(
                            pv0[0:65, n0:512],
                            vext[j][:, 0:65],
                            e[:, n0:512],
                            start=(j == 0), stop=last,
                        )
                        nc.tensor.matmul(
                            pv1[0:65, n0:512],
                            vext[j][:, 65:130],
                            e[:, 512 + n0 : 1024],
                            start=(j == 0), stop=last,
                        )

                    pends = []
                    for j in range(jmax):
                        diag = j * KT >= qi * NQ
                        n0 = j * KT - qi * NQ if diag else 0
                        q0 = qi * NQ
                        sc = pp_sc.tile([128, 1024], F32, name=f"sc_{b}_{qi}_{j}", tag="sc")
                        nc.tensor.matmul(
                            sc[:, n0:512],
                            kTt[0:64, j * KT : (j + 1) * KT],
                            qT[0:64, q0 + n0 : q0 + 512],
                            start=True, stop=True,
                            tile_position=(0, 0),
                        )
                        nc.tensor.matmul(
                            sc[:, 512 + n0 : 1024],
                            kTt[64:128, j * KT : (j + 1) * KT],
                            qT[64:128, q0 + n0 : q0 + 512],
                            start=True, stop=True,
                            tile_position=(64, 0),
                        )
                        e = expt_pool.tile([128, 1024], BF16, name=f"e_{b}_{qi}_{j}", tag="e")
                        sc3 = sc.rearrange("p (s n) -> p s n", s=2)[:, :, n0:512]
                        e3 = e.rearrange("p (s n) -> p s n", s=2)[:, :, n0:512]
                        nc.scalar.activation(
                            e3, sc3,
                            mybir.ActivationFunctionType.Exp, scale=1.0 / np.sqrt(DK),
                        )
                        if diag:
                            em = e.rearrange("p (s n) -> p s n", s=2)[:, :, n0 : n0 + 128]
                            nc.vector.tensor_mul(
                                em, em, tri[:, None, :].broadcast_to([128, 2, 128])
                            )
                        # PV lags two units behind QK/exp so exp latency and
                        # ACT jitter never stall the PE stream
                        if len(pends) == 2:
                            emit_pv(*pends.pop(0))
                        pends.append((j, e, n0))
                        # last batch: ration filler (no gen_A behind it)
                        if b < B - 1 or j % 2 == 1:
                            advance(fillers)
                    for p in pends:
                        emit_pv(*p)
                    # normalize by denominator row
                    for h, pv in ((0, pv0), (1, pv1)):
                        dn = small_pool.tile([1, 512], F32, name=f"dn{h}_{b}_{qi}", tag="dn")
                        nc.vector.tensor_copy(out=dn, in_=pv[64:65, :])
                        rc = small_pool.tile([1, 512], F32, name=f"rc{h}_{b}_{qi}", tag="rc")
                        nc.vector.reciprocal_approx_fast(out=rc, in_=dn)
                        bc = small_pool.tile([64, 512], F32, name=f"bc{h}_{b}_{qi}", tag="bc")
                        nc.gpsimd.partition_broadcast(bc, rc, channels=64)
                        nc.vector.tensor_mul(
                            headsT[64 * h : 64 * h + 64, qi * NQ : (qi + 1) * NQ],
                            pv[0:64, :],
                            bc,
                        )
                    # pull filler matmuls into the PE stream across the
                    # epilogue chain (next q-block's PV waits on a PSUM slot);
                    # append this q-block's projection only afterwards, since
                    # it depends on the epilogue just emitted
                    advance(fillers)
                    advance(fillers)
                    advance(fillers)
                    fillers.append(gen_proj(b, qi))

            # ---- software pipeline across batches ----
            for _ in gen_A(0):
                pass
            fillers = []
            for b in range(B):
                if b + 1 < B:
                    fillers.append(gen_A(b + 1))
                run_C(b, fillers)
                # next batch needs its QKV complete before attention starts
                if b + 1 < B:
                    for g in list(fillers):
                        if g.__name__ == "gen_A":
                            for _ in g:
                                pass
                            fillers.remove(g)
            for g in fillers:
                for _ in g:
                    pass

    nc.compile()
    return nc


def _get_nc():
    if "nc" not in _CACHE:
        _CACHE["nc"] = _build()
    return _CACHE["nc"]


def kernel(x, W_qkv, W_o):
    x = np.asarray(x, dtype=np.float32)
    W_qkv = np.asarray(W_qkv, dtype=np.float32)
    W_o = np.asarray(W_o, dtype=np.float32)

    bf = ml_dtypes.bfloat16
    xt = np.ascontiguousarray(x.transpose(0, 2, 1)).reshape(B, KD, 128, S).astype(bf)
    kk, qq = np.meshgrid(np.arange(128), np.arange(128), indexing="ij")
    tri = (kk <= qq).astype(bf)
    iden = np.eye(128, dtype=bf)

    in_maps = []
    for c in range(NCORES):
        r0 = CH * c
        rows = np.r_[r0 : r0 + CH, D + r0 : D + r0 + CH, 2 * D + r0 : 2 * D + r0 + CH]
        wqkvt = np.ascontiguousarray(W_qkv[rows].T).reshape(KD, 128, 3 * CH).astype(bf)
        wot = np.ascontiguousarray(W_o[:, r0 : r0 + CH].T).astype(bf)
        in_maps.append(
            {"xt": xt, "wqkvt": wqkvt, "wot": wot, "tri": tri, "iden": iden}
        )

    nc = _get_nc()
    res = bass_utils.run_bass_kernel_spmd(nc, in_maps, core_ids=list(range(NCORES)))
    _CACHE["last_result"] = res
    acc = np.zeros((D, B * S), dtype=np.float64)
    for c in range(NCORES):
        acc += res.results[c]["outt"].reshape(D, B * S).astype(np.float64)
    return np.ascontiguousarray(acc.T).reshape(B, S, D).astype(np.float32)


# revision 36
# speedup vs baseline: 1.0447x; 1.0447x over previous
"""Multi-head self-attention (B=4, S=2048, D=1024, H=16, causal) on 8 TRN2
NeuronCores, tensor-parallel over heads (2 heads per core).

Per-core (bf16 matmuls, fp32 PSUM):
  qkv projection (channel-major) -> v PE-transposed to [s,dk]+ones col ->
  attention with TRANSPOSED scores ([k,q] layout; no probs transposes;
  exp on ScalarE over a fused h0|h1 [128,1024] PSUM pair; causal skip +
  triangle mask on diagonal tiles; PV accumulates [65,q]=[v|1]^T@expT,
  row 64 = softmax denominator; normalization on [64,q] head outputs) ->
  output projection partial per q-block; host sums 8 partials.

Emission order software-pipelines: next batch's QKV/v-transposes fill the
attention loop, each q-block's projection follows its epilogue -- keeps
TensorE dense so HAM stays at K=8/8.
"""

import numpy as np
import ml_dtypes

import concourse.bacc as bacc
import concourse.mybir as mybir
import concourse.tile as tile
from concourse import bass_utils

B, S, D, H, DK = 4, 2048, 1024, 16, 64
NCORES = 8
HPC = H // NCORES          # heads per core = 2
CH = HPC * DK              # qkv channels per head group = 128
NQ = 512                   # q block
KT = 128                   # k tile
NQB = S // NQ              # 4 q-blocks per batch
NKT = S // KT              # 16 k-tiles per batch
KD = D // 128              # 8 contraction tiles for projections

F32 = mybir.dt.float32
BF16 = mybir.dt.bfloat16

_CACHE = {}


def _build():
    nc = bacc.Bacc("TRN2", target_bir_lowering=False, debug=False)

    xt_d = nc.dram_tensor("xt", [B, KD, 128, S], BF16, kind="ExternalInput")
    wqkvt_d = nc.dram_tensor("wqkvt", [KD, 128, 3 * CH], BF16, kind="ExternalInput")
    wot_d = nc.dram_tensor("wot", [CH, D], BF16, kind="ExternalInput")
    tri_d = nc.dram_tensor("tri", [128, 128], BF16, kind="ExternalInput")
    iden_d = nc.dram_tensor("iden", [128, 128], BF16, kind="ExternalInput")
    outt_d = nc.dram_tensor("outt", [D // 128, 128, B * S], F32, kind="ExternalOutput")

    with tile.TileContext(nc) as tc:
        with (
            tc.tile_pool(name="xt", bufs=2 * KD) as xt_pool,
            tc.tile_pool(name="qk", bufs=2) as qk_pool,
            tc.tile_pool(name="vst", bufs=2) as vst_pool,
            tc.tile_pool(name="vext", bufs=2 * NKT) as vext_pool,
            tc.tile_pool(name="expt", bufs=6) as expt_pool,
            tc.tile_pool(name="heads", bufs=2) as heads_pool,
            tc.tile_pool(name="wp", bufs=1) as w_pool,
            tc.tile_pool(name="outs", bufs=6) as out_pool,
            tc.tile_pool(name="small", bufs=4) as small_pool,
            tc.tile_pool(name="ppa", bufs=2, space="PSUM") as pp_a,
            tc.tile_pool(name="ppsc", bufs=2, space="PSUM") as pp_sc,
            tc.tile_pool(name="pppv", bufs=2, space="PSUM") as pp_pv,
        ):
            # --- persistent weights / constants ---
            # Only wq[k=0] is DMA'd up front: the first QKV matmul needs just
            # wq[0] + xt[0], so the remaining weight loads are interleaved
            # after the batch-0 x tiles (see gen_A) to cut the startup stall.
            wq = w_pool.tile([128, KD * 3 * CH], BF16, name="wq")
            nc.sync.dma_start(out=wq[:, 0 : 3 * CH], in_=wqkvt_d.ap()[0])
            wot = w_pool.tile([128, D], BF16, name="wot")
            tri = w_pool.tile([128, 128], BF16, name="tri")
            iden = w_pool.tile([128, 128], BF16, name="iden")

            st = {}

            def gen_A(b):
                """QKV projection + v transposes for batch b. Yields ~20x."""
                xts = []
                for k in range(KD):
                    xtk = xt_pool.tile([128, S], BF16, name=f"xt_{b}_{k}", tag="xt")
                    nc.sync.dma_start(out=xtk, in_=xt_d.ap()[b, k])
                    xts.append(xtk)
                    if b == 0 and k < KD - 1:
                        nc.sync.dma_start(
                            out=wq[:, (k + 1) * 3 * CH : (k + 2) * 3 * CH],
                            in_=wqkvt_d.ap()[k + 1],
                        )
                if b == 0:
                    nc.sync.dma_start(out=wot, in_=wot_d.ap())
                    nc.sync.dma_start(out=tri, in_=tri_d.ap())
                    nc.sync.dma_start(out=iden, in_=iden_d.ap())
                qT = qk_pool.tile([128, S], BF16, name=f"q_{b}", tag="q")
                kTt = qk_pool.tile([128, S], BF16, name=f"k_{b}", tag="k")
                vT = vst_pool.tile([128, S], BF16, name=f"v_{b}", tag="v")
                st[b] = {"q": qT, "k": kTt, "v": vT, "ve": []}
                dests = [qT, kTt, vT]
                for m in range(3):
                    for n in range(S // 512):
                        ps = pp_a.tile([128, 512], F32, name=f"qkv_{b}_{m}_{n}", tag="mm")
                        for k in range(KD):
                            nc.tensor.matmul(
                                ps,
                                wq[:, k * 3 * CH + m * CH : k * 3 * CH + (m + 1) * CH],
                                xts[k][:, n * 512 : (n + 1) * 512],
                                start=(k == 0),
                                stop=(k == KD - 1),
                            )
                            if k == KD // 2 - 1:
                                yield
                        dst = dests[m][:, n * 512 : (n + 1) * 512]
                        if n % 2 == 0:
                            nc.scalar.copy(dst, ps)
                        else:
                            nc.vector.tensor_copy(out=dst, in_=ps)
                        yield
                for t in range(NKT):
                    pt = pp_a.tile([128, 128], BF16, name=f"vt_{b}_{t}", tag="mm")
                    nc.tensor.transpose(pt, vT[:, t * 128 : (t + 1) * 128], iden)
                    ve = vext_pool.tile([128, 130], BF16, name=f"vext_{b}_{t}", tag="vext")
                    nc.gpsimd.memset(ve[:, 64:65], 1.0)
                    nc.gpsimd.memset(ve[:, 129:130], 1.0)
                    nc.vector.tensor_copy(out=ve[:, 0:64], in_=pt[:, 0:64])
                    nc.vector.tensor_copy(out=ve[:, 65:129], in_=pt[:, 64:128])
                    st[b]["ve"].append(ve)
                    yield

            def gen_proj(b, n):
                """Output projection for columns [n*512,(n+1)*512) of batch b."""
                headsT = st[b]["h"]
                for m in range(D // 128):
                    ps = pp_a.tile([128, 512], F32, name=f"proj_{b}_{m}_{n}", tag="mm")
                    nc.tensor.matmul(
                        ps,
                        wot[:, m * 128 : (m + 1) * 128],
                        headsT[:, n * 512 : (n + 1) * 512],
                        start=True, stop=True,
                    )
                    ot = out_pool.tile([128, 512], F32, name=f"out_{b}_{m}_{n}", tag="o")
                    if m % 2 == 0:
                        nc.vector.tensor_copy(out=ot, in_=ps)
                    else:
                        nc.scalar.copy(ot, ps)
                    nc.sync.dma_start(
                        out=outt_d.ap()[m, :, b * S + n * 512 : b * S + (n + 1) * 512],
                        in_=ot,
                    )
                    yield

            def advance(fillers):
                for g in list(fillers):
                    try:
                        next(g)
                        return
                    except StopIteration:
                        fillers.remove(g)

            def run_C(b, fillers):
                """Attention for batch b, fillers interleaved per unit."""
                qT, kTt = st[b]["q"], st[b]["k"]
                vext = st[b]["ve"]
                headsT = heads_pool.tile([128, S], BF16, name=f"heads_{b}", tag="h")
                st[b]["h"] = headsT
                for qi in range(NQB):
                    pv0 = pp_pv.tile([128, 512], F32, name=f"pv0_{b}_{qi}", tag="pv")
                    pv1 = pp_pv.tile([128, 512], F32, name=f"pv1_{b}_{qi}", tag="pv")
                    jmax = (qi + 1) * (NQ // KT)

                    def emit_pv(j, e, n0):
                        last = j == jmax - 1
                        nc.tensor.matmul(
                            pv0[0:65, n0:512],
                            vext[j][:, 0:65],
                            e[:, n0:512],
                            start=(j == 0), stop=last,
                        )
                        nc.tensor.matmul(
                            pv1[0:65, n0:512],
                            vext[j][:, 65:130],
                            e[:, 512 + n0 : 1024],
                            start=(j == 0), stop=last,
                        )

                    pends = []
                    for j in range(jmax):
                        diag = j * KT >= qi * NQ
                        n0 = j * KT - qi * NQ if diag else 0
                        q0 = qi * NQ
                        sc = pp_sc.tile([128, 1024], F32, name=f"sc_{b}_{qi}_{j}", tag="sc")
                        nc.tensor.matmul(
                            sc[:, n0:512],
                            kTt[0:64, j * KT : (j + 1) * KT],
                            qT[0:64, q0 + n0 : q0 + 512],
                            start=True, stop=True,
                            tile_position=(0, 0),
                        )
                        nc.tensor.matmul(
                            sc[:, 512 + n0 : 1024],
                            kTt[64:128, j * KT : (j + 1) * KT],
                            qT[64:128, q0 + n0 : q0 + 512],
                            start=True, stop=True,
                            tile_position=(64, 0),
                        )
                        e = expt_pool.tile([128, 1024], BF16, name=f"e_{b}_{qi}_{j}", tag="e")
                        sc3 = sc.rearrange("p (s n) -> p s n", s=2)[:, :, n0:512]
                        e3 = e.rearrange("p (s n) -> p s n", s=2)[:, :, n0:512]
                        nc.scalar.activation(
                            e3, sc3,
                            mybir.ActivationFunctionType.Exp, scale=1.0 / np.sqrt(DK),
                        )
                        if diag:
                            em = e.rearrange("p (s n) -> p s n", s=2)[:, :, n0 : n0 + 128]
                            nc.vector.tensor_mul(
                                em, em, tri[:, None, :].broadcast_to([128, 2, 128])
                            )
                        # PV lags two units behind QK/exp so exp latency and
                        # ACT jitter never stall the PE stream
                        if len(pends) == 2:
                            emit_pv(*pends.pop(0))
                        pends.append((j, e, n0))
                        advance(fillers)
                    for p in pends:
                        emit_pv(*p)
                    # normalize by denominator row
                    for h, pv in ((0, pv0), (1, pv1)):
                        dn = small_pool.tile([1, 512], F32, name=f"dn{h}_{b}_{qi}", tag="dn")
                        nc.vector.tensor_copy(out=dn, in_=pv[64:65, :])
                        rc = small_pool.tile([1, 512], F32, name=f"rc{h}_{b}_{qi}", tag="rc")
                        nc.vector.reciprocal_approx_fast(out=rc, in_=dn)
                        bc = small_pool.tile([64, 512], F32, name=f"bc{h}_{b}_{qi}", tag="bc")
                        nc.gpsimd.partition_broadcast(bc, rc, channels=64)
                        nc.vector.tensor_mul(
                            headsT[64 * h : 64 * h + 64, qi * NQ : (qi + 1) * NQ],
                            pv[0:64, :],
                            bc,
                        )
                    # pull filler matmuls into the PE stream across the
                    # epilogue chain (next q-block's PV waits on a PSUM slot);
                    # append this q-block's projection only afterwards, since
                    # it depends on the epilogue just emitted
                    advance(fillers)
                    advance(fillers)
                    advance(fillers)
                    fillers.append(gen_proj(b, qi))

            # ---- software pipeline across batches ----
            for _ in gen_A(0):
                pass
            fillers = []
            for b in range(B):
                if b + 1 < B:
                    fillers.append(gen_A(b + 1))
                run_C(b, fillers)
                # next batch needs its QKV complete before attention starts
                if b + 1 < B:
                    for g in list(fillers):
                        if g.__name__ == "gen_A":
                            for _ in g:
                                pass
                            fillers.remove(g)
            for g in fillers:
                for _ in g:
                    pass

    nc.compile()
    return nc


def _get_nc():
    if "nc" not in _CACHE:
        _CACHE["nc"] = _build()
    return _CACHE["nc"]


def kernel(x, W_qkv, W_o):
    x = np.asarray(x, dtype=np.float32)
    W_qkv = np.asarray(W_qkv, dtype=np.float32)
    W_o = np.asarray(W_o, dtype=np.float32)

    bf = ml_dtypes.bfloat16
    xt = np.ascontiguousarray(x.transpose(0, 2, 1)).reshape(B, KD, 128, S).astype(bf)
    kk, qq = np.meshgrid(np.arange(128), np.arange(128), indexing="ij")
    tri = (kk <= qq).astype(bf)
    iden = np.eye(128, dtype=bf)

    in_maps = []
    for c in range(NCORES):
        r0 = CH * c
        rows = np.r_[r0 : r0 + CH, D + r0 : D + r0 + CH, 2 * D + r0 : 2 * D + r0 + CH]
        wqkvt = np.ascontiguousarray(W_qkv[rows].T).reshape(KD, 128, 3 * CH).astype(bf)
        wot = np.ascontiguousarray(W_o[:, r0 : r0 + CH].T).astype(bf)
        in_maps.append(
            {"xt": xt, "wqkvt": wqkvt, "wot": wot, "tri": tri, "iden": iden}
        )

    nc = _get_nc()
    res = bass_utils.run_bass_kernel_spmd(nc, in_maps, core_ids=list(range(NCORES)))
    _CACHE["last_result"] = res
    acc = np.zeros((D, B * S), dtype=np.float64)
    for c in range(NCORES):
        acc += res.results[c]["outt"].reshape(D, B * S).astype(np.float64)
    return np.ascontiguousarray(acc.T).reshape(B, S, D).astype(np.float32)
